# revision 14
# baseline (speedup 1.0000x reference)
"""EMA (first-order IIR) forward kernel for Trainium2, SPMD over 8 NeuronCores.

y[b, c, t] = gamma[c] * y[b, c, t-1] + (1 - gamma[c]) * x[b, c, t],  y[.., -1] = 0
gamma = sigmoid(weight)

Sharding: data-parallel over B (8 batches -> 8 cores, zero communication).
Per core: x_shard [C=512, T=8192] with channels SORTED by gamma (host-side
permutation; output rows inverse-permuted on host). Channels go on SBUF
partitions (4 sorted groups of 128).

The recurrence is computed two ways:
 - group 0 (smallest gammas): builtin DVE tensor_tensor_scan
   (state = g*state + x), 2 cycles/element.
 - groups 1-3: a custom DVE op (EMA_ZSCAN_ANT) at 1 element/cycle:
       out[p,k] = (carry + sum_{j<=k} x[p,j]*ramp[p,j]) * g[p]^(k+1),
   with ramp[p,j] = g[p]^-(j+1) precomputed on host. This is exactly
   y_k = g^(k+1)*carry + sum g^(k-j) x_j. fp32 range caps the chunk
   length at N ~ 78/(-ln g); sorting makes per-group caps large.

I/O is bf16 (x downcast, y upcast on host): halves HBM traffic. Each
steady-state DMA spans all 4 groups (1.5-2 MiB) for near-peak HBM rate.
Constants (g, 1-g, ramps) ride in one packed aux tensor -> one DMA.
"""

import os

import numpy as np
import ml_dtypes

import concourse.bass as bass
import concourse.tile as tile
from concourse import bacc, mybir
from concourse.bass_utils import run_bass_kernel_spmd

B, C, T = 8, 512, 8192
P = 128              # SBUF partition count
NG = C // P          # channel groups per core
N_CORES = 8

_sched = os.environ.get("EMA_SCHED", "768,1280,2048,2048,1536,512")
CHUNKS = [int(c) for c in _sched.split(",")]
assert sum(CHUNKS) == T, CHUNKS

MODE = os.environ.get("EMA_MODE", "scan")     # zscan | scan
# Channel groups whose scan runs on GPSIMD (parallel engine) instead of DVE.
GP_GROUPS = {int(s) for s in os.environ.get("EMA_GP_GROUPS", "").split(",") if s}
ZMAX = 512            # cap on z-chunk length (bounds aux width)
ZMIN = 16             # below this, builtin scan is cheaper
RANGE_LOG = 78.0      # fp32 exponent budget for the ramp

XBUFS = int(os.environ.get("EMA_XBUFS", "2"))
YSBUFS = int(os.environ.get("EMA_YSBUFS", "2"))
YOBUFS = int(os.environ.get("EMA_YOBUFS", "2"))

LAST_RESULT = None   # BassKernelResults of the most recent run (for test.py)

_prog_cache = {}
_ZSCAN_OP = None


def _zscan_ref(in0, in1, s0, s1, imm2):
    u = in0.astype(np.float32) * in1.astype(np.float32)
    z = np.asarray(s1, dtype=np.float32).reshape(-1, 1) + np.cumsum(u, axis=-1)
    n = in0.shape[-1]
    r = np.asarray(s0, np.float32).reshape(-1, 1) ** np.arange(1, n + 1, dtype=np.float32)
    return (z * r).astype(np.float32)


def _register_zscan():
    """Register the EMA_ZSCAN_ANT custom DVE op (idempotent)."""
    global _ZSCAN_OP
    if _ZSCAN_OP is not None:
        return _ZSCAN_OP
    from concourse.dve_spec import Spec, Src0, Src1, C0, C1, One, scan, AluOp, lower
    from concourse import dve_ops as _dve_ops
    from concourse.dve_ops import DveOp, has_src1, _SUB_OPCODE_FOR_NAME, OPS
    from concourse.dve_table_gen import DveOpSpec, dve_ver_for

    name = "EMA_ZSCAN_ANT"
    if name in _SUB_OPCODE_FOR_NAME:
        _ZSCAN_OP = next(op for op in OPS if op.name == name)
        return _ZSCAN_OP
    body = scan(AluOp.ADD, Src0 * Src1, init=C1) * scan(AluOp.MULTIPLY, C0, init=One)
    spec = Spec(body=body, reference=_zscan_ref)
    row = max(_SUB_OPCODE_FOR_NAME.values()) + 1
    assert row < 0x20, "custom-DVE row space exhausted"
    shas = {}
    for ver in ("v3", "v4"):
        shas[ver] = DveOpSpec(name=name, opcode=row, uops=lower(spec, ver=ver),
                              rd1_en=has_src1(spec)).sha(ver)
    op = DveOp(name, spec, subdim=False, uops_sha=shas)
    OPS.append(op)
    _SUB_OPCODE_FOR_NAME[name] = row
    _dve_ops.CUSTOM_DVE_SPECS[name] = spec
    _ZSCAN_OP = op
    return op


def _plan_groups(gamma_sorted):
    """Per sorted 128-channel group: z-chunk cap N (0 = use builtin scan)."""
    lam = -np.log(gamma_sorted.astype(np.float64))
    plan = []
    for gi in range(NG):
        lmax = lam[gi * P:(gi + 1) * P].max()
        n = int(RANGE_LOG / max(lmax, 1e-9))
        n = min(n, ZMAX, max(CHUNKS))
        if MODE != "zscan" or n < ZMIN:
            n = 0
        plan.append(n)
    return tuple(plan)


def _build_program(zplan):
    key = (tuple(CHUNKS), zplan, tuple(sorted(GP_GROUPS)), XBUFS, YSBUFS, YOBUFS)
    if key in _prog_cache:
        return _prog_cache[key]
    if any(zplan):
        _register_zscan()

    nc = bacc.Bacc("TRN2", target_bir_lowering=False, debug=False)
    f32 = mybir.dt.float32
    bf16 = mybir.dt.bfloat16

    # aux layout: [0:NG]=g, [NG:2NG]=og, then ramps for z-groups
    roff = {}
    aw = 2 * NG
    for gi, n in enumerate(zplan):
        if n:
            roff[gi] = aw
            aw += n

    x_d = nc.dram_tensor("x", [C, T], bf16, kind="ExternalInput").ap()
    aux_d = nc.dram_tensor("aux", [P, aw], f32, kind="ExternalInput").ap()
    y_d = nc.dram_tensor("y", [C, T], bf16, kind="ExternalOutput").ap()

    xv = x_d.rearrange("(g p) t -> p g t", p=P)
    yv = y_d.rearrange("(g p) t -> p g t", p=P)

    with tile.TileContext(nc) as tc:
        with (
            tc.tile_pool(name="cols", bufs=1) as cols,
            tc.tile_pool(name="xin", bufs=XBUFS) as xp,
            tc.tile_pool(name="ys", bufs=YSBUFS) as ysp,
            tc.tile_pool(name="yo", bufs=YOBUFS) as yop,
        ):
            aux = cols.tile([P, aw], f32, tag="aux")
            nc.sync.dma_start(aux[:], aux_d)
            g_col = [aux[:, gi:gi + 1] for gi in range(NG)]
            og_col = [aux[:, NG + gi:NG + gi + 1] for gi in range(NG)]

            op = _ZSCAN_OP
            # carry state per group: (tile, col) of last produced output col
            prev = [None] * NG

            def emit_group(gi, xt_slice, fk):
                """Emit DVE ops for one group's chunk; returns ys tile.
                bf16 ys is fine for the builtin scan (state is fp32 inside the
                DVE; only the carry re-read rounds). The custom z-op requires
                an f32 carry scalar, so z-groups use f32 ys."""
                ys_dt = f32 if (zplan[gi] or os.environ.get("EMA_YS_DT", "bf16") == "f32") else bf16
                ys = ysp.tile([P, fk], ys_dt, tag=f"ys{gi}")
                n_cap = zplan[gi]
                if not n_cap:
                    init = 0.0 if prev[gi] is None else prev[gi]
                    eng = nc.gpsimd if gi in GP_GROUPS else nc.vector
                    eng.tensor_tensor_scan(
                        ys[:], g_col[gi].broadcast_to([P, fk]), xt_slice, init,
                        mybir.AluOpType.mult, mybir.AluOpType.add,
                    )
                    prev[gi] = ys[:, fk - 1:fk]
                    return ys, None
                # z-op subchunks: return generator state for round-robin
                subs = []
                off = 0
                while off < fk:
                    n = min(n_cap, fk - off)
                    subs.append((off, n))
                    off += n

                def emit_sub(off, n):
                    s1 = 0.0 if prev[gi] is None else prev[gi]
                    nc.vector._custom_dve(
                        op, out=ys[:, off:off + n], in0=xt_slice[:, off:off + n],
                        in1=aux[:, roff[gi]:roff[gi] + n],
                        s0=g_col[gi], s1=s1,
                    )
                    prev[gi] = ys[:, off + n - 1:off + n]
                return ys, (subs, emit_sub)

            span = os.environ.get("EMA_SPAN", "1") == "1"
            t0 = 0
            for ci, fk in enumerate(CHUNKS):
                if ci == 0 or not span:
                    xts = []
                    for gi in range(NG):
                        xg = xp.tile([P, fk], bf16, tag=f"x0{gi}")
                        nc.sync.dma_start(xg[:], xv[:, gi, t0:t0 + fk])
                        xts.append(xg[:])
                else:
                    xt = xp.tile([P, NG, fk], bf16, tag="x")
                    nc.sync.dma_start(xt[:], xv[:, :, t0:t0 + fk])
                    xts = [xt[:, gi] for gi in range(NG)]

                yo = yop.tile([P, NG, fk], bf16, tag="yo")
                pending = []   # (gi, ys, subs, emit_sub)
                for gi in range(NG):
                    ys, zstate = emit_group(gi, xts[gi], fk)
                    if zstate is None:
                        # builtin scan done; scale now
                        nc.scalar.activation(
                            yo[:, gi], ys[:], mybir.ActivationFunctionType.Copy,
                            scale=og_col[gi],
                        )
                    else:
                        pending.append((gi, ys, list(zstate[0]), zstate[1]))
                # round-robin the z subchunks across groups
                while pending:
                    nxt = []
                    for gi, ys, subs, emit_sub in pending:
                        off, n = subs.pop(0)
                        emit_sub(off, n)
                        if subs:
                            nxt.append((gi, ys, subs, emit_sub))
                        else:
                            nc.scalar.activation(
                                yo[:, gi], ys[:],
                                mybir.ActivationFunctionType.Copy,
                                scale=og_col[gi],
                            )
                    pending = nxt
                nc.scalar.dma_start(yv[:, :, t0:t0 + fk], yo[:])
                t0 += fk

    nc.compile()
    _prog_cache[key] = nc
    return nc


def kernel(x: np.ndarray, weight: np.ndarray) -> np.ndarray:
    global LAST_RESULT
    assert x.shape == (B, C, T) and weight.shape == (C,)

    gamma64 = 1.0 / (1.0 + np.exp(-weight.astype(np.float64)))
    perm = np.argsort(gamma64, kind="stable")
    g_s64 = gamma64[perm]
    g_s = g_s64.astype(np.float32)
    og_s = (np.float32(1.0) - g_s).astype(np.float32)

    zplan = _plan_groups(g_s)

    # aux: [P, aw] f32
    aw = 2 * NG + sum(zplan)
    aux = np.zeros((P, aw), dtype=np.float32)
    for gi in range(NG):
        aux[:, gi] = g_s[gi * P:(gi + 1) * P]
        aux[:, NG + gi] = og_s[gi * P:(gi + 1) * P]
    off = 2 * NG
    for gi, n in enumerate(zplan):
        if n:
            gg = g_s64[gi * P:(gi + 1) * P][:, None]
            j = np.arange(1, n + 1, dtype=np.float64)[None, :]
            aux[:, off:off + n] = (gg ** (-j)).astype(np.float32)
            off += n

    x_s = np.ascontiguousarray(
        x[:, perm, :], dtype=np.float32).astype(ml_dtypes.bfloat16)

    nc = _build_program(zplan)
    in_maps = [{"x": x_s[i], "aux": aux} for i in range(N_CORES)]
    trace = os.environ.get("EMA_TRACE", "0") == "1"
    LAST_RESULT = run_bass_kernel_spmd(
        nc, in_maps, list(range(N_CORES)), trace=trace,
    )
    y_sorted = np.stack([LAST_RESULT.results[i]["y"] for i in range(N_CORES)])
    out = np.empty((B, C, T), dtype=np.float32)
    out[:, perm, :] = y_sorted.astype(np.float32)
    return out


# revision 16
# speedup vs baseline: 1.0274x; 1.0274x over previous
"""EMA (first-order IIR) forward kernel for Trainium2, SPMD over 8 NeuronCores.

y[b, c, t] = gamma[c] * y[b, c, t-1] + (1 - gamma[c]) * x[b, c, t],  y[.., -1] = 0
gamma = sigmoid(weight)

Sharding: data-parallel over B (8 batches -> 8 cores, zero communication).
Per core: x_shard [C=512, T=8192] with channels SORTED by gamma (host-side
permutation; output rows inverse-permuted on host). Channels go on SBUF
partitions (4 sorted groups of 128).

The recurrence is computed two ways:
 - group 0 (smallest gammas): builtin DVE tensor_tensor_scan
   (state = g*state + x), 2 cycles/element.
 - groups 1-3: a custom DVE op (EMA_ZSCAN_ANT) at 1 element/cycle:
       out[p,k] = (carry + sum_{j<=k} x[p,j]*ramp[p,j]) * g[p]^(k+1),
   with ramp[p,j] = g[p]^-(j+1) precomputed on host. This is exactly
   y_k = g^(k+1)*carry + sum g^(k-j) x_j. fp32 range caps the chunk
   length at N ~ 78/(-ln g); sorting makes per-group caps large.

I/O is bf16 (x downcast, y upcast on host): halves HBM traffic. Each
steady-state DMA spans all 4 groups (1.5-2 MiB) for near-peak HBM rate.
Constants (g, 1-g, ramps) ride in one packed aux tensor -> one DMA.
"""

import os

import numpy as np
import ml_dtypes

import concourse.bass as bass
import concourse.tile as tile
from concourse import bacc, mybir
from concourse.bass_utils import run_bass_kernel_spmd

B, C, T = 8, 512, 8192
P = 128              # SBUF partition count
NG = C // P          # channel groups per core
N_CORES = 8

_sched = os.environ.get("EMA_SCHED", "768,1280,2048,2048,1536,512")
CHUNKS = [int(c) for c in _sched.split(",")]
assert sum(CHUNKS) == T, CHUNKS

MODE = os.environ.get("EMA_MODE", "scan")     # zscan | scan
# Channel groups whose scan runs on GPSIMD (parallel engine) instead of DVE.
GP_GROUPS = {int(s) for s in os.environ.get("EMA_GP_GROUPS", "").split(",") if s}
ZMAX = 512            # cap on z-chunk length (bounds aux width)
ZMIN = 16             # below this, builtin scan is cheaper
RANGE_LOG = 78.0      # fp32 exponent budget for the ramp

XBUFS = int(os.environ.get("EMA_XBUFS", "2"))
YSBUFS = int(os.environ.get("EMA_YSBUFS", "2"))
YOBUFS = int(os.environ.get("EMA_YOBUFS", "2"))

LAST_RESULT = None   # BassKernelResults of the most recent run (for test.py)

_prog_cache = {}
_ZSCAN_OP = None


def _zscan_ref(in0, in1, s0, s1, imm2):
    u = in0.astype(np.float32) * in1.astype(np.float32)
    z = np.asarray(s1, dtype=np.float32).reshape(-1, 1) + np.cumsum(u, axis=-1)
    n = in0.shape[-1]
    r = np.asarray(s0, np.float32).reshape(-1, 1) ** np.arange(1, n + 1, dtype=np.float32)
    return (z * r).astype(np.float32)


def _register_zscan():
    """Register the EMA_ZSCAN_ANT custom DVE op (idempotent)."""
    global _ZSCAN_OP
    if _ZSCAN_OP is not None:
        return _ZSCAN_OP
    from concourse.dve_spec import Spec, Src0, Src1, C0, C1, One, scan, AluOp, lower
    from concourse import dve_ops as _dve_ops
    from concourse.dve_ops import DveOp, has_src1, _SUB_OPCODE_FOR_NAME, OPS
    from concourse.dve_table_gen import DveOpSpec, dve_ver_for

    name = "EMA_ZSCAN_ANT"
    if name in _SUB_OPCODE_FOR_NAME:
        _ZSCAN_OP = next(op for op in OPS if op.name == name)
        return _ZSCAN_OP
    body = scan(AluOp.ADD, Src0 * Src1, init=C1) * scan(AluOp.MULTIPLY, C0, init=One)
    spec = Spec(body=body, reference=_zscan_ref)
    row = max(_SUB_OPCODE_FOR_NAME.values()) + 1
    assert row < 0x20, "custom-DVE row space exhausted"
    shas = {}
    for ver in ("v3", "v4"):
        shas[ver] = DveOpSpec(name=name, opcode=row, uops=lower(spec, ver=ver),
                              rd1_en=has_src1(spec)).sha(ver)
    op = DveOp(name, spec, subdim=False, uops_sha=shas)
    OPS.append(op)
    _SUB_OPCODE_FOR_NAME[name] = row
    _dve_ops.CUSTOM_DVE_SPECS[name] = spec
    _ZSCAN_OP = op
    return op


def _plan_groups(gamma_sorted):
    """Per sorted 128-channel group: z-chunk cap N (0 = use builtin scan)."""
    lam = -np.log(gamma_sorted.astype(np.float64))
    plan = []
    for gi in range(NG):
        lmax = lam[gi * P:(gi + 1) * P].max()
        n = int(RANGE_LOG / max(lmax, 1e-9))
        n = min(n, ZMAX, max(CHUNKS))
        if MODE != "zscan" or n < ZMIN:
            n = 0
        plan.append(n)
    return tuple(plan)


def _build_program(zplan):
    key = (tuple(CHUNKS), zplan, tuple(sorted(GP_GROUPS)), XBUFS, YSBUFS, YOBUFS)
    if key in _prog_cache:
        return _prog_cache[key]
    if any(zplan):
        _register_zscan()

    nc = bacc.Bacc("TRN2", target_bir_lowering=False, debug=False)
    f32 = mybir.dt.float32
    bf16 = mybir.dt.bfloat16

    # aux layout: [0:NG]=g, [NG:2NG]=og, then ramps for z-groups
    roff = {}
    aw = 2 * NG
    for gi, n in enumerate(zplan):
        if n:
            roff[gi] = aw
            aw += n

    x_d = nc.dram_tensor("x", [C, T], bf16, kind="ExternalInput").ap()
    aux_d = nc.dram_tensor("aux", [P, aw], f32, kind="ExternalInput").ap()
    y_d = nc.dram_tensor("y", [C, T], bf16, kind="ExternalOutput").ap()

    xv = x_d.rearrange("(g p) t -> p g t", p=P)
    yv = y_d.rearrange("(g p) t -> p g t", p=P)

    with tile.TileContext(nc) as tc:
        with (
            tc.tile_pool(name="cols", bufs=1) as cols,
            tc.tile_pool(name="xin", bufs=XBUFS) as xp,
            tc.tile_pool(name="ys", bufs=YSBUFS) as ysp,
            tc.tile_pool(name="yo", bufs=YOBUFS) as yop,
        ):
            aux = cols.tile([P, aw], f32, tag="aux")
            nc.scalar.dma_start(aux[:], aux_d)
            g_col = [aux[:, gi:gi + 1] for gi in range(NG)]
            og_col = [aux[:, NG + gi:NG + gi + 1] for gi in range(NG)]

            op = _ZSCAN_OP
            # carry state per group: (tile, col) of last produced output col
            prev = [None] * NG

            def emit_group(gi, xt_slice, fk):
                """Emit DVE ops for one group's chunk; returns ys tile.
                bf16 ys is fine for the builtin scan (state is fp32 inside the
                DVE; only the carry re-read rounds). The custom z-op requires
                an f32 carry scalar, so z-groups use f32 ys."""
                ys_dt = f32 if (zplan[gi] or os.environ.get("EMA_YS_DT", "f32") == "f32") else bf16
                ys = ysp.tile([P, fk], ys_dt, tag=f"ys{gi}")
                n_cap = zplan[gi]
                if not n_cap:
                    init = 0.0 if prev[gi] is None else prev[gi]
                    eng = nc.gpsimd if gi in GP_GROUPS else nc.vector
                    eng.tensor_tensor_scan(
                        ys[:], g_col[gi].broadcast_to([P, fk]), xt_slice, init,
                        mybir.AluOpType.mult, mybir.AluOpType.add,
                    )
                    prev[gi] = ys[:, fk - 1:fk]
                    return ys, None
                # z-op subchunks: return generator state for round-robin
                subs = []
                off = 0
                while off < fk:
                    n = min(n_cap, fk - off)
                    subs.append((off, n))
                    off += n

                def emit_sub(off, n):
                    s1 = 0.0 if prev[gi] is None else prev[gi]
                    nc.vector._custom_dve(
                        op, out=ys[:, off:off + n], in0=xt_slice[:, off:off + n],
                        in1=aux[:, roff[gi]:roff[gi] + n],
                        s0=g_col[gi], s1=s1,
                    )
                    prev[gi] = ys[:, off + n - 1:off + n]
                return ys, (subs, emit_sub)

            span = os.environ.get("EMA_SPAN", "1") == "1"
            t0 = 0
            for ci, fk in enumerate(CHUNKS):
                if ci == 0 or not span:
                    xts = []
                    for gi in range(NG):
                        xg = xp.tile([P, fk], bf16, tag=f"x0{gi}")
                        nc.sync.dma_start(xg[:], xv[:, gi, t0:t0 + fk])
                        xts.append(xg[:])
                else:
                    xt = xp.tile([P, NG, fk], bf16, tag="x")
                    nc.sync.dma_start(xt[:], xv[:, :, t0:t0 + fk])
                    xts = [xt[:, gi] for gi in range(NG)]

                yo = yop.tile([P, NG, fk], bf16, tag="yo")
                pending = []   # (gi, ys, subs, emit_sub)
                for gi in range(NG):
                    ys, zstate = emit_group(gi, xts[gi], fk)
                    if zstate is None:
                        # builtin scan done; scale now
                        nc.scalar.activation(
                            yo[:, gi], ys[:], mybir.ActivationFunctionType.Copy,
                            scale=og_col[gi],
                        )
                    else:
                        pending.append((gi, ys, list(zstate[0]), zstate[1]))
                # round-robin the z subchunks across groups
                while pending:
                    nxt = []
                    for gi, ys, subs, emit_sub in pending:
                        off, n = subs.pop(0)
                        emit_sub(off, n)
                        if subs:
                            nxt.append((gi, ys, subs, emit_sub))
                        else:
                            nc.scalar.activation(
                                yo[:, gi], ys[:],
                                mybir.ActivationFunctionType.Copy,
                                scale=og_col[gi],
                            )
                    pending = nxt
                nc.scalar.dma_start(yv[:, :, t0:t0 + fk], yo[:])
                t0 += fk

    nc.compile()
    _prog_cache[key] = nc
    return nc


def kernel(x: np.ndarray, weight: np.ndarray) -> np.ndarray:
    global LAST_RESULT
    assert x.shape == (B, C, T) and weight.shape == (C,)

    gamma64 = 1.0 / (1.0 + np.exp(-weight.astype(np.float64)))
    perm = np.argsort(gamma64, kind="stable")
    g_s64 = gamma64[perm]
    g_s = g_s64.astype(np.float32)
    og_s = (np.float32(1.0) - g_s).astype(np.float32)

    zplan = _plan_groups(g_s)

    # aux: [P, aw] f32
    aw = 2 * NG + sum(zplan)
    aux = np.zeros((P, aw), dtype=np.float32)
    for gi in range(NG):
        aux[:, gi] = g_s[gi * P:(gi + 1) * P]
        aux[:, NG + gi] = og_s[gi * P:(gi + 1) * P]
    off = 2 * NG
    for gi, n in enumerate(zplan):
        if n:
            gg = g_s64[gi * P:(gi + 1) * P][:, None]
            j = np.arange(1, n + 1, dtype=np.float64)[None, :]
            aux[:, off:off + n] = (gg ** (-j)).astype(np.float32)
            off += n

    x_s = np.ascontiguousarray(
        x[:, perm, :], dtype=np.float32).astype(ml_dtypes.bfloat16)

    nc = _build_program(zplan)
    in_maps = [{"x": x_s[i], "aux": aux} for i in range(N_CORES)]
    trace = os.environ.get("EMA_TRACE", "0") == "1"
    LAST_RESULT = run_bass_kernel_spmd(
        nc, in_maps, list(range(N_CORES)), trace=trace,
    )
    y_sorted = np.stack([LAST_RESULT.results[i]["y"] for i in range(N_CORES)])
    out = np.empty((B, C, T), dtype=np.float32)
    out[:, perm, :] = y_sorted.astype(np.float32)
    return out
